# revision 6
# baseline (speedup 1.0000x reference)
"""ColorQuantizer (VQ nearest-palette-color) Trainium2 Bass kernel.

Reference semantics: out[b,:,h,w] = palette[argmin_k ||(x+0.01*noise)[b,:,h,w] - palette[k]||]
(The straight-through estimator is numerically the identity on the forward pass.)

Sharding: pure data parallel over batch (32 -> 8 cores x 4), palette replicated.
"""
import sys

sys.path.insert(0, "/opt/trn_rl_repo")

import numpy as np

import concourse.bacc as bacc
import concourse.mybir as mybir
from concourse.tile import TileContext
from concourse.bass_utils import run_bass_kernel_spmd

# Problem constants (hardcoded per harness contract)
B, C, H, W = 32, 3, 512, 512
K = 16
N_CORES = 8
B_PER_CORE = B // N_CORES  # 4
NOISE_SCALE = 0.01

F = 1024          # free-dim elements per tile
HROWS = 256       # h-rows consumed per tile (128 partitions x 2 rows)
T_PER_PLANE = H // HROWS  # 2 tiles per (batch, h) plane split

_DT = mybir.dt.float32


def _plane_ap(t_dram, b, c, t):
    """[128, F] view of channel plane c of batch b, h-rows [t*256,(t+1)*256)."""
    return t_dram[b, c, t * HROWS : (t + 1) * HROWS, :].rearrange(
        "(p a) w -> p (a w)", p=128
    )


def _build(repeat=1):
    nc = bacc.Bacc("TRN2", target_bir_lowering=False, debug=False,
                   num_devices=N_CORES)
    x = nc.dram_tensor("x", [B_PER_CORE, C, H, W], _DT, kind="ExternalInput").ap()
    n = nc.dram_tensor("noise", [B_PER_CORE, C, H, W], _DT, kind="ExternalInput").ap()
    pal = nc.dram_tensor("palette", [K, C], _DT, kind="ExternalInput").ap()
    o = nc.dram_tensor("out", [B_PER_CORE, C, H, W], _DT, kind="ExternalOutput").ap()

    Alu = mybir.AluOpType
    Act = mybir.ActivationFunctionType

    with TileContext(nc) as tc:
        with (
            tc.tile_pool(name="const", bufs=1) as cpool,
            tc.tile_pool(name="io", bufs=3) as io,
            tc.tile_pool(name="scratch", bufs=3) as sc,
            tc.tile_pool(name="carry", bufs=2) as carry,
        ):
            # palette -> SBUF [128, 48] broadcast across partitions; col = k*3+c
            pal_sb = cpool.tile([128, K * C], _DT)
            nc.sync.dma_start(
                out=pal_sb[:],
                in_=pal.rearrange("(o k) c -> o (k c)", o=1).to_broadcast([128, K * C]),
            )
            # negated palette for ACT Square bias
            npal_sb = cpool.tile([128, K * C], _DT)
            nc.vector.tensor_scalar(
                out=npal_sb[:], in0=pal_sb[:], scalar1=-1.0, scalar2=None,
                op0=Alu.mult)

            for rep in range(repeat):
              for b in range(B_PER_CORE):
                for t in range(T_PER_PLANE):
                    xt = [io.tile([128, F], _DT, tag=f"x{c}", name=f"xt{c}") for c in range(C)]
                    nt = [io.tile([128, F], _DT, tag=f"n{c}", name=f"nt{c}") for c in range(C)]
                    for c in range(C):
                        nc.sync.dma_start(out=xt[c][:], in_=_plane_ap(x, b, c, t))
                        nc.sync.dma_start(out=nt[c][:], in_=_plane_ap(n, b, c, t))

                    # y_c = x_c + NOISE_SCALE * n_c
                    yt = [sc.tile([128, F], _DT, tag=f"y{c}", name=f"yt{c}") for c in range(C)]
                    for c in range(C):
                        nc.vector.scalar_tensor_tensor(
                            out=yt[c][:], in0=nt[c][:], scalar=NOISE_SCALE,
                            in1=xt[c][:], op0=Alu.mult, op1=Alu.add)

                    m = carry.tile([128, F], _DT, tag="m")
                    mask = carry.tile([128, F], mybir.dt.uint8, tag="mask")
                    ot = [carry.tile([128, F], _DT, tag=f"o{c}", name=f"ot{c}") for c in range(C)]

                    for k in range(K):
                        q = [sc.tile([128, F], _DT, tag=f"q{c}", name=f"qt{c}") for c in range(C)]
                        for c in range(C):
                            # q_c = (y_c - p_kc)^2
                            nc.scalar.activation(
                                out=q[c][:], in_=yt[c][:], func=Act.Square,
                                bias=npal_sb[:, k * C + c : k * C + c + 1],
                                scale=1.0)
                        if k == 0:
                            # d -> m directly; out_c = palette color 0
                            nc.vector.tensor_tensor(
                                out=m[:], in0=q[0][:], in1=q[1][:], op=Alu.add)
                            nc.vector.tensor_tensor(
                                out=m[:], in0=m[:], in1=q[2][:], op=Alu.add)
                            for c in range(C):
                                nc.vector.tensor_copy(
                                    out=ot[c][:],
                                    in_=pal_sb[:, c : c + 1].to_broadcast([128, F]))
                        else:
                            d = sc.tile([128, F], _DT, tag="d")
                            nc.vector.tensor_tensor(
                                out=d[:], in0=q[0][:], in1=q[1][:], op=Alu.add)
                            nc.vector.tensor_tensor(
                                out=d[:], in0=d[:], in1=q[2][:], op=Alu.add)
                            # strict less => first-wins tie-breaking
                            nc.vector.tensor_tensor(
                                out=mask[:], in0=d[:], in1=m[:], op=Alu.is_lt)
                            nc.vector.tensor_tensor(
                                out=m[:], in0=m[:], in1=d[:], op=Alu.min)
                            for c in range(C):
                                nc.vector.copy_predicated(
                                    out=ot[c][:], mask=mask[:],
                                    data=pal_sb[:, k * C + c : k * C + c + 1]
                                    .to_broadcast([128, F]))

                    for c in range(C):
                        nc.sync.dma_start(out=_plane_ap(o, b, c, t), in_=ot[c][:])

    nc.compile()
    return nc


_NC_CACHE = {}


def _get_nc(repeat=1):
    if repeat not in _NC_CACHE:
        _NC_CACHE[repeat] = _build(repeat)
    return _NC_CACHE[repeat]


def kernel(x, noise, palette):
    x = np.ascontiguousarray(np.asarray(x, dtype=np.float32))
    noise = np.ascontiguousarray(np.asarray(noise, dtype=np.float32))
    palette = np.ascontiguousarray(np.asarray(palette, dtype=np.float32))
    nc = _get_nc()
    in_maps = [
        {
            "x": x[i * B_PER_CORE : (i + 1) * B_PER_CORE],
            "noise": noise[i * B_PER_CORE : (i + 1) * B_PER_CORE],
            "palette": palette,
        }
        for i in range(N_CORES)
    ]
    res = run_bass_kernel_spmd(nc, in_maps, list(range(N_CORES)))
    out = np.concatenate([res.results[i]["out"] for i in range(N_CORES)], axis=0)
    return out.astype(np.float32, copy=False)


if __name__ == "__main__":
    rng = np.random.default_rng(0)
    x = rng.random((B, C, H, W), dtype=np.float32)
    noise = rng.standard_normal((B, C, H, W), dtype=np.float32)
    palette = rng.random((K, C), dtype=np.float32)
    out = kernel(x, noise, palette)
    y = np.transpose(x + NOISE_SCALE * noise, (0, 2, 3, 1)).reshape(-1, 3)
    d = ((y[:, None, :] - palette[None, :, :]) ** 2).sum(-1)
    idx = np.argmin(d, axis=-1)
    expect = np.transpose(
        palette[idx].reshape(B, H, W, C), (0, 3, 1, 2))
    err = np.abs(out - expect).max()
    print("abs max err vs numpy argmin:", err)
    mism = (out != expect).any(axis=1).sum()
    print("mismatched pixels:", mism, "/", B * H * W)
